# revision 16
# baseline (speedup 1.0000x reference)
"""Trainium2 Bass kernel for NeoX-style attention block (B=2, S=2048, D=2048,
H=16, HS=128, partial RoPE rot=32, no mask) sharded over 8 NeuronCores.

Sharding: core c handles batch b = c//4 and head group g = c%4 (4 heads).
Tensor-parallel over heads: W_qkv column-sliced, W_dense row-sliced; each core
produces a partial [S, D] output; host sums 4 partials per batch + bias.

On-chip layout (per core):
  hidden[b] is fed transposed (hT [D, S]) so the QKV projection produces
  qkv^T directly: qkvT[cols, tokens] = W_slice^T @ hT via PE matmuls with
  lhsT = W chunks (natural layout) and rhs = hT chunks. Heads processed in
  2 passes of 2 heads (SBUF budget). Per head: scores^T chunks
  S^T[k,q] = K^T_chunk.T @ Q^T (f32r, N=512), exp on ACT straight from
  PSUM, unnormalized O^T accum = V_chunk.T @ E plus a ones-matmul that
  accumulates the softmax denominators replicated across partitions;
  normalization = DVE reciprocal + multiply. Dense partial uses lhsT = O^T
  (already the right layout), rhs = W_dense row-slice.

All matmuls run in float32r (FP22 truncation, full PE rate at N>=256).
"""
import sys

sys.path.insert(0, "/opt/trn_rl_repo")

import numpy as np
from contextlib import ExitStack

import concourse.bass as bass  # noqa: F401  (registers engine types)
import concourse.tile as tile
from concourse import bacc, mybir
from concourse import bass_utils

F32 = mybir.dt.float32
F32R = mybir.dt.float32r
MUL = mybir.AluOpType.mult
ADD = mybir.AluOpType.add

B, S, D = 2, 2048, 2048
H, HS, ROT = 16, 128, 32
BASE = 10000.0
SM_SCALE = 1.0 / float(np.sqrt(HS))

HPC = 4            # heads per core
CPB = 4            # cores per batch
NCORES = 8
WCOLS = HPC * 3 * HS          # 1536 qkv columns per core
PASS_COLS = WCOLS // 2        # 768 per pass (2 heads)
NQ = 256                      # token slice width in QKV projection
KC = D // 128                 # 16 contraction chunks

_NC = None
TRACE = False
LAST_RESULT = [None]


def _build():
    nc = bacc.Bacc("TRN2", target_bir_lowering=False, debug=False)
    hT = nc.dram_tensor("hT", [D, S], F32, kind="ExternalInput").ap()
    wq = nc.dram_tensor("wq", [D, WCOLS], F32, kind="ExternalInput").ap()
    bq = nc.dram_tensor("bq", [WCOLS, 1], F32, kind="ExternalInput").ap()
    wd = nc.dram_tensor("wd", [HPC * HS, D], F32, kind="ExternalInput").ap()
    tabc = nc.dram_tensor("tabc", [ROT, S], F32, kind="ExternalInput").ap()
    tabs = nc.dram_tensor("tabs", [ROT, S], F32, kind="ExternalInput").ap()
    rotm = nc.dram_tensor("rotm", [ROT, ROT], F32, kind="ExternalInput").ap()
    ones = nc.dram_tensor("ones", [128, 128], F32, kind="ExternalInput").ap()
    ident = nc.dram_tensor("ident", [128, 128], F32, kind="ExternalInput").ap()
    outp = nc.dram_tensor("outp", [S, D], F32, kind="ExternalOutput").ap()

    with tile.TileContext(nc) as tc:
        with ExitStack() as ctx:
            glob = ctx.enter_context(tc.tile_pool(name="glob", bufs=1))
            epool = ctx.enter_context(tc.tile_pool(name="epool", bufs=3))
            vpool = ctx.enter_context(tc.tile_pool(name="vpool", bufs=2))
            ps = ctx.enter_context(tc.tile_pool(name="ps", bufs=1, space="PSUM"))

            # ---- constants / tables (global) ----
            tabc_sb = glob.tile([ROT, S], F32, tag="tabc")
            nc.sync.dma_start(tabc_sb[:], tabc)
            tabs_sb = glob.tile([ROT, S], F32, tag="tabs")
            nc.sync.dma_start(tabs_sb[:], tabs)
            rot_sb = glob.tile([ROT, ROT], F32R, tag="rotm")
            nc.sync.dma_start(rot_sb[:], rotm.bitcast(F32R))
            ones_sb = glob.tile([128, 128], F32R, tag="ones")
            nc.sync.dma_start(ones_sb[:], ones.bitcast(F32R))
            id_sb = glob.tile([128, 128], F32R, tag="ident")
            nc.sync.dma_start(id_sb[:], ident.bitcast(F32R))
            bq_sb = glob.tile([128, 12], F32, tag="bq")
            nc.sync.dma_start(
                bq_sb[:].rearrange("p (m o) -> p m o", m=12),
                bq.rearrange("(m p) o -> p m o", p=128),
            )

            # persistent activations
            qkvT = glob.tile([128, 6 * S], F32R, tag="qkvT")   # reused per pass
            oT = glob.tile([128, HPC * S], F32R, tag="oT")

            with ExitStack() as s1:
                wpool = s1.enter_context(tc.tile_pool(name="wpool", bufs=1))
                hpool = s1.enter_context(tc.tile_pool(name="hpool", bufs=2))

                def rope_one(m, n4):
                    # rotate first ROT dims of the q/k chunk at column m for
                    # the 512-token window n4 (runs on DVE; overlaps QKV MMs)
                    base = m * S
                    sl = slice(base + n4 * 512, base + (n4 + 1) * 512)
                    pr = ps.tile([128, 512], F32, tag="u", bufs=4, name=f"pr{m}_{n4}")
                    nc.tensor.matmul(pr[0:ROT, :], rot_sb[:, :],
                                     qkvT[0:ROT, sl], start=True, stop=True)
                    nc.vector.tensor_tensor(
                        pr[0:ROT, :], pr[0:ROT, :],
                        tabs_sb[:, n4 * 512:(n4 + 1) * 512], op=MUL)
                    nc.vector.tensor_tensor(
                        qkvT[0:ROT, sl], qkvT[0:ROT, sl],
                        tabc_sb[:, n4 * 512:(n4 + 1) * 512], op=MUL)
                    nc.vector.tensor_tensor(
                        qkvT[0:ROT, sl], qkvT[0:ROT, sl], pr[0:ROT, :], op=ADD)

                def qkv_pass(hg):
                    w_sb = wpool.tile([128, KC * PASS_COLS], F32R, tag="w")
                    wsrc = (wq[:, hg * PASS_COLS:(hg + 1) * PASS_COLS]
                            .rearrange("(kc p) m -> kc p m", p=128).bitcast(F32R))
                    # column-pair-major so the first m-pair's weights land
                    # before the rest of W (cuts the cold-start DMA wait)
                    for pi, pair in enumerate(((0, 1), (3, 4), (2, 5))):
                        for kc in range(KC):
                            for m in pair:
                                nc.sync.dma_start(
                                    w_sb[:, kc * PASS_COLS + m * 128:kc * PASS_COLS + (m + 1) * 128],
                                    wsrc[kc, :, m * 128:(m + 1) * 128])
                    for n in range(S // NQ):
                        ht = hpool.tile([128, KC * NQ], F32R, tag="ht")
                        hsrc = hT[:, n * NQ:(n + 1) * NQ].rearrange(
                            "(kc p) s -> kc p s", p=128).bitcast(F32R)
                        for kc in range(KC):
                            nc.sync.dma_start(ht[:, kc * NQ:(kc + 1) * NQ], hsrc[kc])
                        # two m-chunk accumulations share one PSUM bank
                        # (sequential groups; a start=True only clears bits,
                        # the finished first half's data is unaffected)
                        for pi, pair in enumerate(((0, 1), (3, 4), (2, 5))):
                            pq = ps.tile([128, 2 * NQ], F32, tag="u", bufs=4, name=f"pq{hg}_{n}_{pi}")
                            for j, m in enumerate(pair):
                                for kc in range(KC):
                                    nc.tensor.matmul(
                                        pq[:, j * NQ:(j + 1) * NQ],
                                        w_sb[:, kc * PASS_COLS + m * 128:kc * PASS_COLS + (m + 1) * 128],
                                        ht[:, kc * NQ:(kc + 1) * NQ],
                                        start=(kc == 0), stop=(kc == KC - 1),
                                    )
                                mg = hg * 6 + m
                                dst = qkvT[:, m * S + n * NQ: m * S + (n + 1) * NQ]
                                if m % 3 == 0:  # q chunk: fold softmax scale
                                    nc.vector.tensor_scalar(
                                        dst, pq[:, j * NQ:(j + 1) * NQ], SM_SCALE,
                                        bq_sb[:, mg:mg + 1], op0=MUL, op1=ADD,
                                    )
                                else:
                                    nc.vector.tensor_scalar_add(
                                        dst, pq[:, j * NQ:(j + 1) * NQ], bq_sb[:, mg:mg + 1])
                        if n % 2 == 1:
                            # q/k columns for this 512-token window done: rope
                            # now so the DVE chain overlaps the next n's MMs
                            for rm in (0, 1, 3, 4):
                                rope_one(rm, n // 2)

                def attention_head(hg, hl):
                    ch = hg * 2 + hl          # core-local head index 0..3
                    qb, kb, vb = (hl * 3) * S, (hl * 3 + 1) * S, (hl * 3 + 2) * S
                    vn = vpool.tile([128, S], F32R, tag="vn", name=f"vn{ch}")
                    for kc in range(KC):
                        pv = ps.tile([128, 128], F32R, tag="u", bufs=4, name=f"pv{ch}_{kc}")
                        nc.tensor.transpose(
                            pv[:], qkvT[:, vb + kc * 128:vb + (kc + 1) * 128], id_sb[:])
                        nc.vector.tensor_copy(vn[:, kc * 128:(kc + 1) * 128], pv[:])
                    # software pipeline carried across q-slices: consumers run
                    # one exp behind producers so neither PE nor ACT restarts
                    pas = {}
                    prev = None

                    def consume(qs, k2, e):
                        po, pm = pas[qs][:, 0:512], pas[qs][:, 512:1024]
                        for j in range(2):
                            kc = 2 * k2 + j
                            ej = e[:, j * 512:(j + 1) * 512]
                            nc.tensor.matmul(po, vn[:, kc * 128:(kc + 1) * 128],
                                             ej, start=(kc == 0), stop=(kc == KC - 1))
                        for j in range(2):
                            kc = 2 * k2 + j
                            ej = e[:, j * 512:(j + 1) * 512]
                            nc.tensor.matmul(pm, ones_sb[:], ej,
                                             start=(kc == 0), stop=(kc == KC - 1))
                        if k2 == KC // 2 - 1:
                            rc = epool.tile([128, 512], F32, tag="e", name=f"rc{ch}_{qs}")
                            nc.vector.reciprocal(rc[:], pm)
                            nc.vector.tensor_tensor(
                                oT[:, ch * S + qs * 512:ch * S + (qs + 1) * 512],
                                po, rc[:], op=MUL)
                            del pas[qs]

                    for qs in range(4):
                        pas[qs] = ps.tile([128, 1024], F32, tag="u", bufs=4, name=f"pa{ch}_{qs}")
                        for k2 in range(KC // 2):
                            pS = ps.tile([128, 1024], F32, tag="u", bufs=4, name=f"pS{ch}_{qs}_{k2}")
                            for j in range(2):
                                kc = 2 * k2 + j
                                nc.tensor.matmul(
                                    pS[:, j * 512:(j + 1) * 512],
                                    qkvT[:, kb + kc * 128:kb + (kc + 1) * 128],
                                    qkvT[:, qb + qs * 512:qb + (qs + 1) * 512],
                                    start=True, stop=True)
                            e = epool.tile([128, 1024], F32R, tag="e")
                            nc.scalar.activation(e[:], pS[:],
                                                 mybir.ActivationFunctionType.Exp)
                            if prev is not None:
                                consume(*prev)
                            prev = (qs, k2, e)
                    consume(*prev)

                qkv_pass(0)
                attention_head(0, 0)
                attention_head(0, 1)
                qkv_pass(1)

            with ExitStack() as s2:
                wdpool = s2.enter_context(tc.tile_pool(name="wdpool", bufs=1))
                bpool = s2.enter_context(tc.tile_pool(name="bpool", bufs=3))
                wd_sb = wdpool.tile([128, HPC * D], F32R, tag="wd")
                wdsrc = wd.rearrange("(hc p) d -> hc p d", p=128).bitcast(F32R)
                for hc in range(HPC):
                    nc.sync.dma_start(wd_sb[:, hc * D:(hc + 1) * D], wdsrc[hc])
                attention_head(1, 0)
                attention_head(1, 1)

                for tt in range(S // 128):
                    for dsp in range(D // 1024):
                        pd = ps.tile([128, 1024], F32, tag="u", bufs=4, name=f"pd{tt}_{dsp}")
                        for j in range(2):
                            ds = 2 * dsp + j
                            for hc in range(HPC):
                                nc.tensor.matmul(
                                    pd[:, j * 512:(j + 1) * 512],
                                    oT[:, hc * S + tt * 128:hc * S + (tt + 1) * 128],
                                    wd_sb[:, hc * D + ds * 512:hc * D + (ds + 1) * 512],
                                    start=(hc == 0), stop=(hc == HPC - 1))
                        bt = bpool.tile([128, 1024], F32, tag="bo")
                        if dsp % 2 == 0:
                            nc.scalar.copy(bt[:], pd[:])
                        else:
                            nc.vector.tensor_copy(bt[:], pd[:])
                        nc.sync.dma_start(
                            outp[tt * 128:(tt + 1) * 128, dsp * 1024:(dsp + 1) * 1024],
                            bt[:])
    nc.compile()
    return nc


def _rope_tables(position_ids_b):
    pos = np.asarray(position_ids_b, dtype=np.float64)
    inv_freq = 1.0 / (BASE ** (np.arange(0, ROT, 2, dtype=np.float64) / ROT))
    freqs = np.outer(pos, inv_freq)                       # [S, 16]
    emb = np.concatenate([freqs, freqs], axis=-1)         # [S, 32]
    return (np.cos(emb).T.astype(np.float32).copy(),
            np.sin(emb).T.astype(np.float32).copy())


def kernel(hidden_states, position_ids, W_qkv, b_qkv, W_dense, b_dense):
    global _NC
    if _NC is None:
        _NC = _build()
    nc = _NC

    hidden_states = np.asarray(hidden_states, dtype=np.float32)
    W_qkv = np.asarray(W_qkv, dtype=np.float32)
    b_qkv = np.asarray(b_qkv, dtype=np.float32)
    W_dense = np.asarray(W_dense, dtype=np.float32)
    b_dense = np.asarray(b_dense, dtype=np.float32)

    rotm = np.zeros((ROT, ROT), np.float32)
    half = ROT // 2
    for i in range(half):
        rotm[i + half, i] = -1.0
        rotm[i, i + half] = 1.0
    ones = np.ones((128, 128), np.float32)
    ident = np.eye(128, dtype=np.float32)

    hTs = [np.ascontiguousarray(hidden_states[b].T) for b in range(B)]
    tabs_per_b = [_rope_tables(np.asarray(position_ids)[b]) for b in range(B)]

    in_maps = []
    for c in range(NCORES):
        b, g = divmod(c, CPB)
        cols = slice(g * WCOLS, (g + 1) * WCOLS)
        bqs = b_qkv[cols].astype(np.float32).copy().reshape(WCOLS, 1)
        # pre-scale q-column biases (scale folded into q at copy time)
        for h in range(HPC):
            bqs[h * 3 * HS:h * 3 * HS + HS] *= SM_SCALE
        cosT, sinT = tabs_per_b[b]
        in_maps.append({
            "hT": hTs[b],
            "wq": np.ascontiguousarray(W_qkv[:, cols]),
            "bq": bqs,
            "wd": np.ascontiguousarray(W_dense[g * HPC * HS:(g + 1) * HPC * HS, :]),
            "tabc": cosT,
            "tabs": sinT,
            "rotm": rotm,
            "ones": ones,
            "ident": ident,
        })

    res = bass_utils.run_bass_kernel_spmd(
        nc, in_maps, core_ids=list(range(NCORES)), trace=TRACE)
    LAST_RESULT[0] = res

    out = np.empty((B, S, D), np.float32)
    for b in range(B):
        acc = np.zeros((S, D), np.float64)
        for g in range(CPB):
            acc += res.results[b * CPB + g]["outp"]
        out[b] = (acc + b_dense).astype(np.float32)
    return out


# revision 17
# speedup vs baseline: 1.0940x; 1.0940x over previous
"""Trainium2 Bass kernel for NeoX-style attention block (B=2, S=2048, D=2048,
H=16, HS=128, partial RoPE rot=32, no mask) sharded over 8 NeuronCores.

Sharding: core c handles batch b = c//4 and head group g = c%4 (4 heads).
Tensor-parallel over heads: W_qkv column-sliced, W_dense row-sliced; each core
produces a partial [S, D] output; host sums 4 partials per batch + bias.

On-chip layout (per core):
  hidden[b] is fed transposed (hT [D, S]) so the QKV projection produces
  qkv^T directly: qkvT[cols, tokens] = W_slice^T @ hT via PE matmuls with
  lhsT = W chunks (natural layout) and rhs = hT chunks. Heads processed in
  2 passes of 2 heads (SBUF budget). Per head: scores^T chunks
  S^T[k,q] = K^T_chunk.T @ Q^T (f32r, N=512), exp on ACT straight from
  PSUM, unnormalized O^T accum = V_chunk.T @ E plus a ones-matmul that
  accumulates the softmax denominators replicated across partitions;
  normalization = DVE reciprocal + multiply. Dense partial uses lhsT = O^T
  (already the right layout), rhs = W_dense row-slice.

All matmuls run in float32r (FP22 truncation, full PE rate at N>=256).
"""
import sys

sys.path.insert(0, "/opt/trn_rl_repo")

import numpy as np
from contextlib import ExitStack

import concourse.bass as bass  # noqa: F401  (registers engine types)
import concourse.tile as tile
from concourse import bacc, mybir
from concourse import bass_utils

F32 = mybir.dt.float32
F32R = mybir.dt.float32r
MUL = mybir.AluOpType.mult
ADD = mybir.AluOpType.add

B, S, D = 2, 2048, 2048
H, HS, ROT = 16, 128, 32
BASE = 10000.0
SM_SCALE = 1.0 / float(np.sqrt(HS))

HPC = 4            # heads per core
CPB = 4            # cores per batch
NCORES = 8
WCOLS = HPC * 3 * HS          # 1536 qkv columns per core
PASS_COLS = WCOLS // 2        # 768 per pass (2 heads)
NQ = 256                      # token slice width in QKV projection
KC = D // 128                 # 16 contraction chunks

_NC = None
TRACE = False
LAST_RESULT = [None]


def _build():
    nc = bacc.Bacc("TRN2", target_bir_lowering=False, debug=False)
    hT = nc.dram_tensor("hT", [D, S], F32, kind="ExternalInput").ap()
    wq = nc.dram_tensor("wq", [D, WCOLS], F32, kind="ExternalInput").ap()
    bq = nc.dram_tensor("bq", [WCOLS, 1], F32, kind="ExternalInput").ap()
    wd = nc.dram_tensor("wd", [HPC * HS, D], F32, kind="ExternalInput").ap()
    tabc = nc.dram_tensor("tabc", [ROT, S], F32, kind="ExternalInput").ap()
    tabs = nc.dram_tensor("tabs", [ROT, S], F32, kind="ExternalInput").ap()
    rotm = nc.dram_tensor("rotm", [ROT, ROT], F32, kind="ExternalInput").ap()
    ones = nc.dram_tensor("ones", [128, 128], F32, kind="ExternalInput").ap()
    ident = nc.dram_tensor("ident", [128, 128], F32, kind="ExternalInput").ap()
    outp = nc.dram_tensor("outp", [S, D], F32, kind="ExternalOutput").ap()

    with tile.TileContext(nc) as tc:
        with ExitStack() as ctx:
            glob = ctx.enter_context(tc.tile_pool(name="glob", bufs=1))
            epool = ctx.enter_context(tc.tile_pool(name="epool", bufs=3))
            vpool = ctx.enter_context(tc.tile_pool(name="vpool", bufs=2))
            ps = ctx.enter_context(tc.tile_pool(name="ps", bufs=1, space="PSUM"))

            # ---- constants / tables (global) ----
            tabc_sb = glob.tile([ROT, S], F32, tag="tabc")
            nc.sync.dma_start(tabc_sb[:], tabc)
            tabs_sb = glob.tile([ROT, S], F32, tag="tabs")
            nc.sync.dma_start(tabs_sb[:], tabs)
            rot_sb = glob.tile([ROT, ROT], F32R, tag="rotm")
            nc.sync.dma_start(rot_sb[:], rotm.bitcast(F32R))
            ones_sb = glob.tile([128, 128], F32R, tag="ones")
            nc.sync.dma_start(ones_sb[:], ones.bitcast(F32R))
            id_sb = glob.tile([128, 128], F32R, tag="ident")
            nc.sync.dma_start(id_sb[:], ident.bitcast(F32R))
            bq_sb = glob.tile([128, 12], F32, tag="bq")
            nc.sync.dma_start(
                bq_sb[:].rearrange("p (m o) -> p m o", m=12),
                bq.rearrange("(m p) o -> p m o", p=128),
            )

            # persistent activations
            qkvT = glob.tile([128, 6 * S], F32R, tag="qkvT")   # reused per pass
            oT = glob.tile([128, HPC * S], F32R, tag="oT")

            with ExitStack() as s1:
                wpool = s1.enter_context(tc.tile_pool(name="wpool", bufs=1))
                hpool = s1.enter_context(tc.tile_pool(name="hpool", bufs=2))

                def rope_one(m, n4):
                    # rotate first ROT dims of the q/k chunk at column m for
                    # the 512-token window n4 (runs on DVE; overlaps QKV MMs)
                    base = m * S
                    sl = slice(base + n4 * 512, base + (n4 + 1) * 512)
                    pr = ps.tile([128, 512], F32, tag="u", bufs=4, name=f"pr{m}_{n4}")
                    nc.tensor.matmul(pr[0:ROT, :], rot_sb[:, :],
                                     qkvT[0:ROT, sl], start=True, stop=True)
                    nc.vector.tensor_tensor(
                        pr[0:ROT, :], pr[0:ROT, :],
                        tabs_sb[:, n4 * 512:(n4 + 1) * 512], op=MUL)
                    nc.vector.tensor_tensor(
                        qkvT[0:ROT, sl], qkvT[0:ROT, sl],
                        tabc_sb[:, n4 * 512:(n4 + 1) * 512], op=MUL)
                    nc.vector.tensor_tensor(
                        qkvT[0:ROT, sl], qkvT[0:ROT, sl], pr[0:ROT, :], op=ADD)

                def qkv_pass(hg):
                    w_sb = wpool.tile([128, KC * PASS_COLS], F32R, tag="w")
                    wsrc = (wq[:, hg * PASS_COLS:(hg + 1) * PASS_COLS]
                            .rearrange("(kc p) m -> kc p m", p=128).bitcast(F32R))
                    for kc in range(KC):
                        nc.sync.dma_start(
                            w_sb[:, kc * PASS_COLS:(kc + 1) * PASS_COLS], wsrc[kc])
                    for n in range(S // NQ):
                        ht = hpool.tile([128, KC * NQ], F32R, tag="ht")
                        hsrc = hT[:, n * NQ:(n + 1) * NQ].rearrange(
                            "(kc p) s -> kc p s", p=128).bitcast(F32R)
                        for kc in range(KC):
                            nc.sync.dma_start(ht[:, kc * NQ:(kc + 1) * NQ], hsrc[kc])
                        # two m-chunk accumulations share one PSUM bank
                        # (sequential groups; a start=True only clears bits,
                        # the finished first half's data is unaffected)
                        for pi, pair in enumerate(((0, 1), (3, 4), (2, 5))):
                            pq = ps.tile([128, 2 * NQ], F32, tag="u", bufs=4, name=f"pq{hg}_{n}_{pi}")
                            for j, m in enumerate(pair):
                                for kc in range(KC):
                                    nc.tensor.matmul(
                                        pq[:, j * NQ:(j + 1) * NQ],
                                        w_sb[:, kc * PASS_COLS + m * 128:kc * PASS_COLS + (m + 1) * 128],
                                        ht[:, kc * NQ:(kc + 1) * NQ],
                                        start=(kc == 0), stop=(kc == KC - 1),
                                    )
                                mg = hg * 6 + m
                                dst = qkvT[:, m * S + n * NQ: m * S + (n + 1) * NQ]
                                if m % 3 == 0:  # q chunk: fold softmax scale
                                    nc.vector.tensor_scalar(
                                        dst, pq[:, j * NQ:(j + 1) * NQ], SM_SCALE,
                                        bq_sb[:, mg:mg + 1], op0=MUL, op1=ADD,
                                    )
                                else:
                                    nc.vector.tensor_scalar_add(
                                        dst, pq[:, j * NQ:(j + 1) * NQ], bq_sb[:, mg:mg + 1])
                        if n % 2 == 1:
                            # q/k columns for this 512-token window done: rope
                            # now so the DVE chain overlaps the next n's MMs
                            for rm in (0, 1, 3, 4):
                                rope_one(rm, n // 2)

                def attention_head(hg, hl):
                    ch = hg * 2 + hl          # core-local head index 0..3
                    qb, kb, vb = (hl * 3) * S, (hl * 3 + 1) * S, (hl * 3 + 2) * S
                    vn = vpool.tile([128, S], F32R, tag="vn", name=f"vn{ch}")
                    for kc in range(KC):
                        pv = ps.tile([128, 128], F32R, tag="u", bufs=4, name=f"pv{ch}_{kc}")
                        nc.tensor.transpose(
                            pv[:], qkvT[:, vb + kc * 128:vb + (kc + 1) * 128], id_sb[:])
                        nc.vector.tensor_copy(vn[:, kc * 128:(kc + 1) * 128], pv[:])
                    # software pipeline carried across q-slices: consumers run
                    # one exp behind producers so neither PE nor ACT restarts
                    pas = {}
                    prev = None

                    def consume(qs, k2, e):
                        po, pm = pas[qs][:, 0:512], pas[qs][:, 512:1024]
                        for j in range(2):
                            kc = 2 * k2 + j
                            ej = e[:, j * 512:(j + 1) * 512]
                            nc.tensor.matmul(po, vn[:, kc * 128:(kc + 1) * 128],
                                             ej, start=(kc == 0), stop=(kc == KC - 1))
                        for j in range(2):
                            kc = 2 * k2 + j
                            ej = e[:, j * 512:(j + 1) * 512]
                            nc.tensor.matmul(pm, ones_sb[:], ej,
                                             start=(kc == 0), stop=(kc == KC - 1))
                        if k2 == KC // 2 - 1:
                            rc = epool.tile([128, 512], F32, tag="e", name=f"rc{ch}_{qs}")
                            nc.vector.reciprocal(rc[:], pm)
                            nc.vector.tensor_tensor(
                                oT[:, ch * S + qs * 512:ch * S + (qs + 1) * 512],
                                po, rc[:], op=MUL)
                            del pas[qs]

                    for qs in range(4):
                        pas[qs] = ps.tile([128, 1024], F32, tag="u", bufs=4, name=f"pa{ch}_{qs}")
                        for k2 in range(KC // 2):
                            pS = ps.tile([128, 1024], F32, tag="u", bufs=4, name=f"pS{ch}_{qs}_{k2}")
                            for j in range(2):
                                kc = 2 * k2 + j
                                nc.tensor.matmul(
                                    pS[:, j * 512:(j + 1) * 512],
                                    qkvT[:, kb + kc * 128:kb + (kc + 1) * 128],
                                    qkvT[:, qb + qs * 512:qb + (qs + 1) * 512],
                                    start=True, stop=True)
                            e = epool.tile([128, 1024], F32R, tag="e")
                            nc.scalar.activation(e[:], pS[:],
                                                 mybir.ActivationFunctionType.Exp)
                            if prev is not None:
                                consume(*prev)
                            prev = (qs, k2, e)
                    consume(*prev)

                qkv_pass(0)
                attention_head(0, 0)
                attention_head(0, 1)
                qkv_pass(1)

            with ExitStack() as s2:
                wdpool = s2.enter_context(tc.tile_pool(name="wdpool", bufs=1))
                bpool = s2.enter_context(tc.tile_pool(name="bpool", bufs=3))
                wd_sb = wdpool.tile([128, HPC * D], F32R, tag="wd")
                wdsrc = wd.rearrange("(hc p) d -> hc p d", p=128).bitcast(F32R)
                for hc in range(HPC):
                    nc.sync.dma_start(wd_sb[:, hc * D:(hc + 1) * D], wdsrc[hc])
                attention_head(1, 0)
                attention_head(1, 1)

                for tt in range(S // 128):
                    for dsp in range(D // 1024):
                        pd = ps.tile([128, 1024], F32, tag="u", bufs=4, name=f"pd{tt}_{dsp}")
                        for j in range(2):
                            ds = 2 * dsp + j
                            for hc in range(HPC):
                                nc.tensor.matmul(
                                    pd[:, j * 512:(j + 1) * 512],
                                    oT[:, hc * S + tt * 128:hc * S + (tt + 1) * 128],
                                    wd_sb[:, hc * D + ds * 512:hc * D + (ds + 1) * 512],
                                    start=(hc == 0), stop=(hc == HPC - 1))
                        bt = bpool.tile([128, 1024], F32, tag="bo")
                        if dsp % 2 == 0:
                            nc.scalar.copy(bt[:], pd[:])
                        else:
                            nc.vector.tensor_copy(bt[:], pd[:])
                        nc.sync.dma_start(
                            outp[tt * 128:(tt + 1) * 128, dsp * 1024:(dsp + 1) * 1024],
                            bt[:])
    nc.compile()
    return nc


def _rope_tables(position_ids_b):
    pos = np.asarray(position_ids_b, dtype=np.float64)
    inv_freq = 1.0 / (BASE ** (np.arange(0, ROT, 2, dtype=np.float64) / ROT))
    freqs = np.outer(pos, inv_freq)                       # [S, 16]
    emb = np.concatenate([freqs, freqs], axis=-1)         # [S, 32]
    return (np.cos(emb).T.astype(np.float32).copy(),
            np.sin(emb).T.astype(np.float32).copy())


def kernel(hidden_states, position_ids, W_qkv, b_qkv, W_dense, b_dense):
    global _NC
    if _NC is None:
        _NC = _build()
    nc = _NC

    hidden_states = np.asarray(hidden_states, dtype=np.float32)
    W_qkv = np.asarray(W_qkv, dtype=np.float32)
    b_qkv = np.asarray(b_qkv, dtype=np.float32)
    W_dense = np.asarray(W_dense, dtype=np.float32)
    b_dense = np.asarray(b_dense, dtype=np.float32)

    rotm = np.zeros((ROT, ROT), np.float32)
    half = ROT // 2
    for i in range(half):
        rotm[i + half, i] = -1.0
        rotm[i, i + half] = 1.0
    ones = np.ones((128, 128), np.float32)
    ident = np.eye(128, dtype=np.float32)

    hTs = [np.ascontiguousarray(hidden_states[b].T) for b in range(B)]
    tabs_per_b = [_rope_tables(np.asarray(position_ids)[b]) for b in range(B)]

    in_maps = []
    for c in range(NCORES):
        b, g = divmod(c, CPB)
        cols = slice(g * WCOLS, (g + 1) * WCOLS)
        bqs = b_qkv[cols].astype(np.float32).copy().reshape(WCOLS, 1)
        # pre-scale q-column biases (scale folded into q at copy time)
        for h in range(HPC):
            bqs[h * 3 * HS:h * 3 * HS + HS] *= SM_SCALE
        cosT, sinT = tabs_per_b[b]
        in_maps.append({
            "hT": hTs[b],
            "wq": np.ascontiguousarray(W_qkv[:, cols]),
            "bq": bqs,
            "wd": np.ascontiguousarray(W_dense[g * HPC * HS:(g + 1) * HPC * HS, :]),
            "tabc": cosT,
            "tabs": sinT,
            "rotm": rotm,
            "ones": ones,
            "ident": ident,
        })

    res = bass_utils.run_bass_kernel_spmd(
        nc, in_maps, core_ids=list(range(NCORES)), trace=TRACE)
    LAST_RESULT[0] = res

    out = np.empty((B, S, D), np.float32)
    for b in range(B):
        acc = np.zeros((S, D), np.float64)
        for g in range(CPB):
            acc += res.results[b * CPB + g]["outp"]
        out[b] = (acc + b_dense).astype(np.float32)
    return out
